# revision 5
# baseline (speedup 1.0000x reference)
"""BinaryConv2d (3x3, SAME, NHWC) Trainium2 Bass kernel — v2.

Strategy (v2: flat-padded full-128 pixel blocks):
  - Data-parallel over batch: 32 images -> 8 cores x 4 images. Weights/bias
    replicated. No collectives.
  - Host prep (tiny): Wq = sign(W) cast to bf16 (+-1 exact) as [cin, 9, cout];
    bias replicated to [128, cout] f32.
  - Spatial layout: each image is a flat 1D signal with rows padded to
    WP=113 (one shared zero column per row boundary). SAME-pad top/bottom
    rows come from zeroed margins of the SBUF image tile. Every 3x3 tap is
    then a pure shift: tap (dh,dw) of center p reads flat p + (dh-1)*113 +
    (dw-1). An output block of M=128 *consecutive* flat centers therefore
    has all 9 lhsT slices contiguous -> full 128-wide stationary operand
    (vs 112/128 in v1), 912k PE cycles/core instead of 1.026M.
  - Per core, per image:
      1. SWDGE cast-DMA x rows f32 NHWC -> bf16 DRAM scratch [112, 113, cin]
         (pad col 112 left as garbage -- zeroed later in SBUF, not DRAM,
         to keep the first transpose off the slow 256B-RMW pad DMA).
      2. HWDGE xbar transpose-DMA 16-row chunks [(16*113), cin] ->
         xt[cin, 128 + 113*r0 : +1808] inside one big per-image SBUF tile
         [cin, 12944] (margins [0,128) and [12784,12944) memset to zero);
         one strided DVE memset per chunk then zeroes the 16 pad-col
         entries the transpose brought in as garbage. Range-granular tile
         deps let matmuls start as soon as the chunks they touch have
         landed, even for reads spanning chunk boundaries.
      3. 99 blocks of 128 centers: 9 accumulating matmuls into PSUM
         [128, 256]; lhsT = xt[:, 128+p0+delta : +128] (stationary, full
         128 pixels), rhs = Wq[:, tap, :] (streaming, cout free dim).
      4. DVE tensor_add(psum, bias) -> bf16 staging [128, 4, 256]; one
         HWDGE store per 4 blocks to out[img, p0:p0+512, :] (bf16).
  - Host epilogue: out [4, 12672, 256] bf16 -> slice [:12656] ->
    [112, 113, 256][:, :112] -> f32. (Drops the pad columns; upcast adds
    ~2e-3 rel err on top of bf16 matmul's 1.7e-3 -- well under the gate.)
"""

import numpy as np

N_CORES = 8
H = 112
W_DIM = 112
CIN = 128
COUT = 256
BATCH = 32
IMG_PER_CORE = BATCH // N_CORES

WP = 113  # padded row stride (one shared zero col per row boundary)
FLAT = H * WP  # 12656 flat positions per image (valid centers: col < 112)
MBLK = 128  # centers per matmul block (full PSUM partition dim)
NBLK = -(-FLAT // MBLK)  # 99 blocks; last block partially garbage
OUT_FLAT = NBLK * MBLK  # 12672 rows in the padded output scratch
MARG_L = 128  # left zero margin (>=114, multiple of 16 for xbar alignment)
MARG_R = 160  # right zero margin (covers last block's max tap read)
XT_LEN = MARG_L + FLAT + MARG_R  # 12944 bf16 cols per partition (25.9 KiB)
RC = 16  # rows per cast/transpose chunk ((RC*WP) % 16 == 0 required)
SB = 4  # output blocks batched per store DMA


def _build_program(n_img, h, w, cin, cout):
    import bass_rust
    import concourse.bacc as bacc
    import concourse.mybir as mybir
    import concourse.tile as tile

    f32 = mybir.dt.float32
    bf16 = mybir.dt.bfloat16

    nc = bacc.Bacc(
        "TRN2", target_bir_lowering=False, debug=False, num_devices=N_CORES
    )
    x_d = nc.dram_tensor("x", [n_img, h, w, cin], f32, kind="ExternalInput").ap()
    w_d = nc.dram_tensor("w", [cin, 9, cout], bf16, kind="ExternalInput").ap()
    b_d = nc.dram_tensor("b", [128, cout], f32, kind="ExternalInput").ap()
    out_d = nc.dram_tensor(
        "out", [n_img, OUT_FLAT, cout], bf16, kind="ExternalOutput"
    ).ap()

    assert h % RC == 0 and (RC * WP) % 16 == 0
    n_chunks = h // RC
    # tap flat-offsets: (dh-1)*WP + (dw-1)
    taps = [(dh - 1) * WP + (dw - 1) for dh in (0, 1, 2) for dw in (0, 1, 2)]

    with tile.TileContext(nc) as tc:
        with (
            tc.tile_pool(name="consts", bufs=1) as cpool,
            tc.tile_pool(name="scratch", bufs=n_img, space="DRAM") as dpool,
            tc.tile_pool(name="xt", bufs=n_img) as xtpool,
            tc.tile_pool(name="psum", bufs=8, space="PSUM") as pspool,
            tc.tile_pool(name="outs", bufs=4) as opool,
        ):
            # consts ride the scalar (ACT HWDGE) ring, which is idle at
            # start -- keeps the sync ring free for the first transpose
            w_t = cpool.tile([cin, 9, cout], bf16)
            nc.scalar.dma_start(out=w_t[:], in_=w_d[:])
            b_t = cpool.tile([128, cout], f32)
            nc.scalar.dma_start(out=b_t[:], in_=b_d[:])

            xts = []
            transpose_insts = []
            PACE = 3  # cast for chunk g waits on transpose g-PACE (keeps the
            # SDMA fabric from flooding with casts and starving the
            # transposes the PE is actually waiting for)

            def prep_image(img):
                scr = dpool.tile([h, WP, cin], bf16, tag="scr")
                xt = xtpool.tile([cin, XT_LEN], bf16, tag="xt")
                nc.vector.memset(xt[:, 0:MARG_L], 0.0)
                nc.vector.memset(xt[:, MARG_L + FLAT : XT_LEN], 0.0)
                # row-structured view of the interior, for pad-col memsets
                xt_rows = xt[:, MARG_L : MARG_L + FLAT].rearrange(
                    "p (r c) -> p r c", c=WP
                )
                for c in range(n_chunks):
                    r0 = c * RC
                    # f32 -> bf16 cast during DMA (SWDGE only)
                    cast = nc.gpsimd.dma_start(
                        out=scr[r0 : r0 + RC, 0:w, :],
                        in_=x_d[img, r0 : r0 + RC],
                    )
                    g = len(transpose_insts)
                    if g >= PACE:
                        bass_rust.add_dep_helper(
                            cast.ins,
                            transpose_insts[g - PACE].ins,
                            sync=True,
                            reason="pace casts behind transposes",
                        )
                    tr = nc.sync.dma_start(
                        out=xt[:, MARG_L + WP * r0 : MARG_L + WP * (r0 + RC)],
                        in_=scr[r0 : r0 + RC].rearrange("a b c -> (a b) c"),
                        transpose=True,
                    )
                    transpose_insts.append(tr)
                    # zero the 16 pad-col entries (transpose brought garbage
                    # from the unwritten scratch col 112)
                    nc.vector.memset(
                        xt_rows[:, r0 : r0 + RC, w : WP], 0.0
                    )
                xts.append(xt)

            # issue ALL input prep up front: range-granular deps let matmuls
            # start as soon as the ranges they read have landed, while the
            # rest streams in behind.
            for img in range(n_img):
                prep_image(img)

            for img in range(n_img):
                xt = xts[img]
                for bq in range(0, NBLK, SB):
                    nb = min(SB, NBLK - bq)
                    ot = opool.tile([MBLK, SB, cout], bf16)
                    for j in range(nb):
                        p0 = (bq + j) * MBLK
                        ps = pspool.tile([MBLK, cout], f32)
                        for k, d in enumerate(taps):
                            nc.tensor.matmul(
                                ps[:],
                                xt[:, MARG_L + p0 + d : MARG_L + p0 + d + MBLK],
                                w_t[:, k, :],
                                start=(k == 0),
                                stop=(k == 8),
                            )
                        nc.vector.tensor_add(ot[:, j, :], ps[:], b_t[:])
                    nc.scalar.dma_start(
                        out=out_d[
                            img, bq * MBLK : (bq + nb) * MBLK
                        ].rearrange("(j p) c -> p j c", j=nb),
                        in_=ot[:, 0:nb, :],
                    )

    nc.compile()
    return nc


_cached_nc = None


def _get_program():
    global _cached_nc
    if _cached_nc is None:
        _cached_nc = _build_program(IMG_PER_CORE, H, W_DIM, CIN, COUT)
    return _cached_nc


def _prep_inputs(x, W, b):
    import ml_dtypes

    # sign with sign(0)=0, matching jnp.sign; bf16 holds +-1/0 exactly
    wq = np.sign(W.astype(np.float32)).astype(ml_dtypes.bfloat16)
    # [3,3,cin,cout] -> [cin, 9, cout]
    wq = np.ascontiguousarray(wq.transpose(2, 0, 1, 3).reshape(CIN, 9, COUT))
    b_rep = np.ascontiguousarray(
        np.broadcast_to(b.astype(np.float32), (128, COUT))
    )
    in_maps = []
    for c in range(N_CORES):
        xs = np.ascontiguousarray(
            x[c * IMG_PER_CORE : (c + 1) * IMG_PER_CORE].astype(np.float32)
        )
        in_maps.append({"x": xs, "w": wq, "b": b_rep})
    return in_maps


def run(x, W, b, trace=False, tmpdir=None):
    from concourse import bass_utils

    if trace:
        # the agent image's antenv lacks axon_hooks; wire the NTFF profile
        # hook up manually so trace=True yields exec_time_ns + pftrace
        import sys, types

        if "antenv.axon_hooks" not in sys.modules:
            import antenv
            from trn_agent_boot.trn_boot import _ntff_profile_via_ctypes

            mod = types.ModuleType("antenv.axon_hooks")
            _hook = _ntff_profile_via_ctypes("/opt/axon/libaxon_pjrt.so")
            mod.get_axon_ntff_profile_hook = lambda: _hook
            sys.modules["antenv.axon_hooks"] = mod
            antenv.axon_hooks = mod

    nc = _get_program()
    in_maps = _prep_inputs(x, W, b)
    res = bass_utils.run_bass_kernel_spmd(
        nc, in_maps, list(range(N_CORES)), trace=trace, tmpdir=tmpdir
    )
    out = np.empty((BATCH, H, W_DIM, COUT), dtype=np.float32)
    for c in range(N_CORES):
        o = res.results[c]["out"]  # [4, OUT_FLAT, 256] bf16
        o = o[:, :FLAT, :].reshape(IMG_PER_CORE, H, WP, COUT)[:, :, :W_DIM, :]
        out[c * IMG_PER_CORE : (c + 1) * IMG_PER_CORE] = o  # upcast + compact
    return out, res


def kernel(x, W, b):
    out, _ = run(x, W, b, trace=False)
    return out
